# revision 5
# baseline (speedup 1.0000x reference)
"""Trainium2 Bass kernel for per-sample modulated causal Conv3D.

Reference semantics (see problem):
  w[b,o,i,kt,kh,kw] = W[o,i,kt,kh,kw] * (cond[b,i] + 1)
  w /= sqrt(max(sum_{i,kt,kh,kw} w^2, eps))        (per b,o)
  y[b] = conv3d(pad(fmap[b], t:(2,0), h:(1,1), w:(1,1)), w[b])  VALID

Shapes: fmap [4,128,16,64,64] f32, cond [4,128], W [128,128,3,3,3].

Strategy: weight modulation/demodulation is a tiny O(1.7MB) elementwise
computation done on host in float64; the 232 GFLOP conv runs on 8
NeuronCores. Sharding: 8 shards = (batch b in 0..3) x (time half in 0..1),
each core computes its sample's full 128 output channels for 8 output
frames. The conv is expressed as 27 accumulated matmuls per PSUM bank:
stationary operand = per-tap modulated weights [i=128, o=128], moving
operand = spatially-padded input frame window [i=128, 8 rows, 64 cols]
(N=512 = one PSUM bank).

Matmul inputs use dtype float32r (full PE rate at N>=512 vs 1/4 rate for
plain float32).
"""

import numpy as np
import orjson

import concourse.bass as bass
import concourse.mybir as mybir
import concourse.tile as tile
from concourse.bass_utils import run_bass_kernel_spmd
from concourse.vector_clock import ScopedClock, VectorClock

# Problem constants (hardcoded per harness contract).
B = 4
DIM = 128
DIM_OUT = 128
T = 16
H = 64
W = 64
TK = 3
SK = 3
EPS = 1e-8
N_CORES = 8
TH = T // 2  # frames per core (time half)
PF = TH + TK - 1  # padded frames per core
HP = H + 2  # spatially padded rows
WP = W + 2  # spatially padded cols
ROWS_PER_BANK = 8  # 8 rows x 64 cols = 512 = one fp32 PSUM bank
N_BANKS = H // ROWS_PER_BANK
BANK_GROUP = 4  # banks accumulated concurrently per group

DT_MM = mybir.dt.float32r  # matmul input dtype (4-byte storage, fp32 data)


class _TileContextChunkedDrain(tile.TileContext):
    """TileContext whose tail drain splits its sem waits across a chain of
    sync NOPs (1 wait each): the walrus build in this container rejects CTRL
    instructions carrying more than one sync-wait command."""

    def _drain_and_barrier(self, tick_clock, wait_clock):
        vec = list(tick_clock.global_clock)
        for i, t in enumerate(vec):
            if t <= 0:
                continue
            v = [0] * len(vec)
            v[i] = t
            nop = self.nc.sync.nop()
            wait_clock.add_sem_waits(nop.ins, ScopedClock({None: VectorClock(v)}))
        self.nc.sync.drain()
        self.nc.all_engine_barrier()
        assert self.sems is not None
        popped = self.nc._tile_sem_poison_stack.pop()
        assert popped is self._sem_poison
        self.nc.clear_and_free_semaphores(list(self.sems.allocated().values()))
        self.nc.all_engine_barrier()


def _split_multi_waits(bir: bytes) -> bytes:
    """The walrus build here rejects instructions carrying more than one
    sync-wait command. Un-fuse: move extra waits onto NoOps inserted just
    before the instruction on the same engine queue (strictly equivalent —
    the engine queue stalls on the NoOp's wait first)."""
    m = orjson.loads(bir)
    ctr = 0
    for f in m["functions"]:
        for bb in f["blocks"]:
            out = []
            for inst in bb["instructions"]:
                si = inst.get("sync_info")
                waits = (si or {}).get("on_wait") or []
                if len(waits) > 1:
                    for w in waits[:-1]:
                        ctr += 1
                        out.append({
                            "debug": inst.get("debug", 0),
                            "engine": inst["engine"],
                            "ins": [],
                            "outs": [],
                            "name": f"I-wsplit{ctr}",
                            "opcode": "NoOp",
                            "sync_info": {"on_update": [], "on_wait": [w]},
                        })
                    si["on_wait"] = [waits[-1]]
                out.append(inst)
            bb["instructions"] = out
    return orjson.dumps(m)


def _patch_to_json_bytes(nc):
    """Route every serialization of this module through _split_multi_waits."""
    orig = nc.to_json_bytes

    def to_json_bytes():
        return _split_multi_waits(orig())

    nc.to_json_bytes = to_json_bytes
    return nc


def build_nc():
    """Build the per-core SPMD Bass program (identical on all cores)."""
    nc = bass.Bass("TRN2", target_bir_lowering=False, debug=False,
                   num_devices=N_CORES)
    xp = nc.dram_tensor("xp", [DIM, PF, HP, WP], DT_MM, kind="ExternalInput")
    wm = nc.dram_tensor("wm", [DIM, TK * SK * SK, DIM_OUT], DT_MM,
                        kind="ExternalInput")
    y = nc.dram_tensor("y", [DIM_OUT, TH, H, W], mybir.dt.float32,
                       kind="ExternalOutput")

    with _TileContextChunkedDrain(nc) as tc:
        with (
            tc.tile_pool(name="wpool", bufs=1) as wpool,
            tc.tile_pool(name="xpool", bufs=4) as xpool,
            tc.tile_pool(name="ypool", bufs=2) as ypool,
            tc.tile_pool(name="ppool", bufs=2 * BANK_GROUP, space="PSUM") as ppool,
        ):
            wt = wpool.tile([DIM, TK * SK * SK, DIM_OUT], DT_MM)
            nc.sync.dma_start(wt[:], wm.ap()[:])

            frames = {}

            def load_frame(fi):
                xt = xpool.tile([DIM, HP, WP], DT_MM, tag="xframe")
                nc.sync.dma_start(xt[:], xp.ap()[:, fi])
                frames[fi] = xt

            for fi in range(TK - 1):
                load_frame(fi)

            for f in range(TH):
                load_frame(f + TK - 1)
                yt = ypool.tile([DIM_OUT, H, W], mybir.dt.float32, tag="yframe")
                for g in range(N_BANKS // BANK_GROUP):
                    pts = [
                        ppool.tile([DIM_OUT, ROWS_PER_BANK, W], mybir.dt.float32,
                                   name=f"ps_f{f}g{g}b{jj}", tag="psum")
                        for jj in range(BANK_GROUP)
                    ]
                    for kt in range(TK):
                        xt = frames[f + kt]
                        for kh in range(SK):
                            for kw in range(SK):
                                tap = kt * SK * SK + kh * SK + kw
                                for jj in range(BANK_GROUP):
                                    j = g * BANK_GROUP + jj
                                    r0 = ROWS_PER_BANK * j + kh
                                    nc.tensor.matmul(
                                        pts[jj][:],
                                        wt[:, tap],
                                        xt[:, r0:r0 + ROWS_PER_BANK, kw:kw + W],
                                        start=(tap == 0),
                                        stop=(tap == TK * SK * SK - 1),
                                    )
                    for jj in range(BANK_GROUP):
                        j = g * BANK_GROUP + jj
                        nc.vector.tensor_copy(
                            yt[:, ROWS_PER_BANK * j:ROWS_PER_BANK * (j + 1), :],
                            pts[jj][:],
                        )
                nc.sync.dma_start(y.ap()[:, f], yt[:])
                del frames[f]
    return _patch_to_json_bytes(nc)


def modulate_weights(weights, cond):
    """Host-side weight modulation + demodulation (float64, cast to f32)."""
    w = weights.astype(np.float64)[None] * (cond.astype(np.float64)[:, None, :,
                                                                    None, None,
                                                                    None] + 1.0)
    ssq = np.sum(w * w, axis=(2, 3, 4, 5), keepdims=True)
    w = w / np.sqrt(np.maximum(ssq, EPS))
    return w.astype(np.float32)  # [B, O, I, TK, SK, SK]


def prepare_inputs(fmap, cond, weights):
    """Shard full inputs into per-core input maps."""
    fmap = np.asarray(fmap, dtype=np.float32)
    cond = np.asarray(cond, dtype=np.float32)
    weights = np.asarray(weights, dtype=np.float32)

    wmod = modulate_weights(weights, cond)
    # [B, O, I, kt, kh, kw] -> [B, I, (kt kh kw), O] so lhsT[i, tap, o].
    wmod = np.ascontiguousarray(wmod.transpose(0, 2, 3, 4, 5, 1).reshape(
        B, DIM, TK * SK * SK, DIM_OUT))

    xpad = np.zeros((B, DIM, T + TK - 1, HP, WP), np.float32)
    xpad[:, :, TK - 1:, 1:1 + H, 1:1 + W] = fmap

    in_maps = []
    for c in range(N_CORES):
        b, th = divmod(c, 2)
        t0 = th * TH
        in_maps.append({
            "xp": np.ascontiguousarray(xpad[b, :, t0:t0 + PF]),
            "wm": wmod[b],
        })
    return in_maps


def assemble_output(results):
    """Gather per-core [O, TH, H, W] results into [B, O, T, H, W]."""
    out = np.empty((B, DIM_OUT, T, H, W), np.float32)
    for c, r in enumerate(results):
        b, th = divmod(c, 2)
        out[b, :, th * TH:(th + 1) * TH] = r["y"]
    return out


_NC_CACHE = []


def kernel(**inputs):
    fmap = inputs["fmap"]
    cond = inputs["cond"]
    weights = inputs["weights"]
    in_maps = prepare_inputs(fmap, cond, weights)
    if not _NC_CACHE:
        _NC_CACHE.append(build_nc())
    nc = _NC_CACHE[0]
    res = run_bass_kernel_spmd(nc, in_maps, core_ids=list(range(N_CORES)))
    return assemble_output(res.results)


# revision 18
# speedup vs baseline: 224.3607x; 224.3607x over previous
"""Trainium2 Bass kernel for per-sample modulated causal Conv3D.

Reference semantics (see problem):
  w[b,o,i,kt,kh,kw] = W[o,i,kt,kh,kw] * (cond[b,i] + 1)
  w /= sqrt(max(sum_{i,kt,kh,kw} w^2, eps))        (per b,o)
  y[b] = conv3d(pad(fmap[b], t:(2,0), h:(1,1), w:(1,1)), w[b])  VALID

Shapes: fmap [4,128,16,64,64] f32, cond [4,128], W [128,128,3,3,3].

Strategy: weight modulation/demodulation is a tiny O(1.7MB) elementwise
computation done on host in float64; the 232 GFLOP conv runs on 8
NeuronCores. Sharding: 8 shards = (batch b in 0..3) x (time half in 0..1),
each core computes its sample's full 128 output channels for 8 output
frames. The conv is expressed as 27 accumulated matmuls per PSUM bank:
stationary operand = per-tap modulated weights [i=128, o=128], moving
operand = spatially-padded input frame window [i=128, 8 rows, 64 cols]
(N=512 = one PSUM bank).

Matmul inputs use dtype float32r (full PE rate at N>=512 vs 1/4 rate for
plain float32).
"""

import numpy as np
import orjson

import concourse.bass as bass
import concourse.mybir as mybir
import concourse.tile as tile
from concourse.bass_utils import run_bass_kernel_spmd
from concourse.vector_clock import ScopedClock, VectorClock

# Problem constants (hardcoded per harness contract).
B = 4
DIM = 128
DIM_OUT = 128
T = 16
H = 64
W = 64
TK = 3
SK = 3
EPS = 1e-8
N_CORES = 8
TH = T // 2  # frames per core (time half)
PF = TH + TK - 1  # padded frames per core
HP = H + 2  # spatially padded rows
WP = W + 2  # spatially padded cols
ROWS_PER_BANK = 8  # 8 rows x 64 cols = 512 = one fp32 PSUM bank
N_BANKS = H // ROWS_PER_BANK
BANK_GROUP = 4  # banks accumulated concurrently per group

DT_MM = mybir.dt.float32r  # matmul input dtype (4-byte storage, fp32 data)


class _TileContextChunkedDrain(tile.TileContext):
    """TileContext whose tail drain splits its sem waits across a chain of
    sync NOPs (1 wait each): the walrus build in this container rejects CTRL
    instructions carrying more than one sync-wait command."""

    def _drain_and_barrier(self, tick_clock, wait_clock):
        vec = list(tick_clock.global_clock)
        for i, t in enumerate(vec):
            if t <= 0:
                continue
            v = [0] * len(vec)
            v[i] = t
            nop = self.nc.sync.nop()
            wait_clock.add_sem_waits(nop.ins, ScopedClock({None: VectorClock(v)}))
        self.nc.sync.drain()
        self.nc.all_engine_barrier()
        assert self.sems is not None
        popped = self.nc._tile_sem_poison_stack.pop()
        assert popped is self._sem_poison
        self.nc.clear_and_free_semaphores(list(self.sems.allocated().values()))
        self.nc.all_engine_barrier()


def _split_multi_waits(bir: bytes) -> bytes:
    """The walrus build here rejects instructions carrying more than one
    sync-wait command. Un-fuse: move extra waits onto NoOps inserted just
    before the instruction on the same engine queue (strictly equivalent —
    the engine queue stalls on the NoOp's wait first)."""
    m = orjson.loads(bir)
    ctr = 0
    for f in m["functions"]:
        for bb in f["blocks"]:
            out = []
            for inst in bb["instructions"]:
                si = inst.get("sync_info")
                waits = (si or {}).get("on_wait") or []
                if len(waits) > 1:
                    for w in waits[:-1]:
                        ctr += 1
                        out.append({
                            "debug": inst.get("debug", 0),
                            "engine": inst["engine"],
                            "ins": [],
                            "outs": [],
                            "name": f"I-wsplit{ctr}",
                            "opcode": "NoOp",
                            "sync_info": {"on_update": [], "on_wait": [w]},
                        })
                    si["on_wait"] = [waits[-1]]
                out.append(inst)
            bb["instructions"] = out
    return orjson.dumps(m)


def _patch_to_json_bytes(nc):
    """Route every serialization of this module through _split_multi_waits."""
    orig = nc.to_json_bytes

    def to_json_bytes():
        return _split_multi_waits(orig())

    nc.to_json_bytes = to_json_bytes
    return nc


def build_nc(repeats=1, bank_group=BANK_GROUP, warmup_mms=24, xbufs=4):
    """Build the per-core SPMD Bass program (identical on all cores).

    repeats>1 re-runs the whole body (loads + conv + stores) that many
    times — used only for slope-based HW timing (axon hides NTFF profiling,
    so per-NEFF wall-clock can't resolve sub-ms kernel times)."""
    nc = bass.Bass("TRN2", target_bir_lowering=False, debug=False,
                   num_devices=N_CORES)
    xp = nc.dram_tensor("xp", [DIM, PF, HP, WP], DT_MM, kind="ExternalInput")
    wm = nc.dram_tensor("wm", [DIM, TK * SK * SK, DIM_OUT], DT_MM,
                        kind="ExternalInput")
    y = nc.dram_tensor("y", [DIM_OUT, TH, H, W], mybir.dt.float32,
                       kind="ExternalOutput")

    with _TileContextChunkedDrain(nc) as tc:
        psum_bufs = min(8, 2 * bank_group)
        with (
            tc.tile_pool(name="wpool", bufs=1) as wpool,
            tc.tile_pool(name="xpool", bufs=xbufs) as xpool,
            tc.tile_pool(name="ypool", bufs=2) as ypool,
            tc.tile_pool(name="ppool", bufs=psum_bufs, space="PSUM") as ppool,
        ):
            if warmup_mms:
                # Keep the PE busy during the initial DMA window so the HAM
                # clock gate reaches 8/8 before the real matmuls start.
                wu_w = wpool.tile([DIM, DIM_OUT], mybir.dt.float32, name="wu_w")
                wu_x = wpool.tile([DIM, ROWS_PER_BANK * W], mybir.dt.float32,
                                  name="wu_x")
                nc.gpsimd.memset(wu_w[:], 0.0)
                nc.gpsimd.memset(wu_x[:], 0.0)
                wu_ps = ppool.tile([DIM_OUT, ROWS_PER_BANK * W],
                                   mybir.dt.float32, name="wu_ps", tag="psum")
                for i in range(warmup_mms):
                    nc.tensor.matmul(wu_ps[:], wu_w[:].bitcast(DT_MM),
                                     wu_x[:].bitcast(DT_MM),
                                     start=True, stop=True)

            wt = wpool.tile([DIM, TK * SK * SK, DIM_OUT], DT_MM)
            frames = {}

            def load_frame(fi, rep=0, chunks=1):
                xt = xpool.tile([DIM, HP, WP], DT_MM, name=f"x_r{rep}f{fi}",
                                tag="xframe")
                if chunks == 1:
                    nc.sync.dma_start(xt[:], xp.ap()[:, fi])
                else:
                    # startup-critical loads: row-chunks across parallel
                    # DMA queues to cut first-matmul latency
                    step = (HP + chunks - 1) // chunks
                    for r0 in range(0, HP, step):
                        r1 = min(r0 + step, HP)
                        nc.sync.dma_start(xt[:, r0:r1], xp.ap()[:, fi, r0:r1])
                frames[fi] = xt

            # Startup order matters: the first matmuls gate on wt[kt=0] and
            # frame 0, so interleave their chunks across the first DMA queue
            # slots; everything else queues behind them.
            xt0 = xpool.tile([DIM, HP, WP], DT_MM, name="x_r0f0", tag="xframe")
            for c in range(SK):
                ws = slice(c * SK, (c + 1) * SK)
                nc.sync.dma_start(wt[:, ws], wm.ap()[:, ws])
                r0, r1 = c * 22, (c + 1) * 22
                nc.sync.dma_start(xt0[:, r0:r1], xp.ap()[:, 0, r0:r1])
            frames[0] = xt0
            for kt in range(1, TK):
                s = slice(kt * SK * SK, (kt + 1) * SK * SK)
                nc.sync.dma_start(wt[:, s], wm.ap()[:, s])

            for rep in range(repeats):
              for fi in range(TK - 1):
                if (rep, fi) == (0, 0):
                    continue  # issued above
                load_frame(fi, rep, chunks=4 if rep == 0 else 1)

              for f in range(TH):
                load_frame(f + TK - 1, rep,
                           chunks=4 if (rep == 0 and f == 0) else 1)
                yt = ypool.tile([DIM_OUT, H, W], mybir.dt.float32, tag="yframe")
                for g in range(N_BANKS // bank_group):
                    pts = [
                        ppool.tile([DIM_OUT, ROWS_PER_BANK, W], mybir.dt.float32,
                                   name=f"ps_r{rep}f{f}g{g}b{jj}", tag="psum")
                        for jj in range(bank_group)
                    ]
                    for kt in range(TK):
                        xt = frames[f + kt]
                        for kh in range(SK):
                            for kw in range(SK):
                                tap = kt * SK * SK + kh * SK + kw
                                for jj in range(bank_group):
                                    j = g * bank_group + jj
                                    r0 = ROWS_PER_BANK * j + kh
                                    nc.tensor.matmul(
                                        pts[jj][:],
                                        wt[:, tap],
                                        xt[:, r0:r0 + ROWS_PER_BANK, kw:kw + W],
                                        start=(tap == 0),
                                        stop=(tap == TK * SK * SK - 1),
                                    )
                    for jj in range(bank_group):
                        j = g * bank_group + jj
                        nc.vector.tensor_copy(
                            yt[:, ROWS_PER_BANK * j:ROWS_PER_BANK * (j + 1), :],
                            pts[jj][:],
                        )
                    # store per bank-group so the kernel tail only waits on
                    # the last group's rows, not a whole frame; the very last
                    # group stores per-bank to shrink the tail further
                    last = (rep == repeats - 1 and f == TH - 1
                            and g == N_BANKS // bank_group - 1)
                    if last:
                        for jj in range(bank_group):
                            j = g * bank_group + jj
                            br = slice(ROWS_PER_BANK * j, ROWS_PER_BANK * (j + 1))
                            nc.sync.dma_start(y.ap()[:, f, br], yt[:, br])
                    else:
                        gr = slice(ROWS_PER_BANK * g * bank_group,
                                   ROWS_PER_BANK * (g + 1) * bank_group)
                        nc.sync.dma_start(y.ap()[:, f, gr], yt[:, gr])
                del frames[f]
    return _patch_to_json_bytes(nc)


def modulate_weights(weights, cond):
    """Host-side weight modulation + demodulation (float64, cast to f32)."""
    w = weights.astype(np.float64)[None] * (cond.astype(np.float64)[:, None, :,
                                                                    None, None,
                                                                    None] + 1.0)
    ssq = np.sum(w * w, axis=(2, 3, 4, 5), keepdims=True)
    w = w / np.sqrt(np.maximum(ssq, EPS))
    return w.astype(np.float32)  # [B, O, I, TK, SK, SK]


def prepare_inputs(fmap, cond, weights):
    """Shard full inputs into per-core input maps."""
    fmap = np.asarray(fmap, dtype=np.float32)
    cond = np.asarray(cond, dtype=np.float32)
    weights = np.asarray(weights, dtype=np.float32)

    wmod = modulate_weights(weights, cond)
    # [B, O, I, kt, kh, kw] -> [B, I, (kt kh kw), O] so lhsT[i, tap, o].
    wmod = np.ascontiguousarray(wmod.transpose(0, 2, 3, 4, 5, 1).reshape(
        B, DIM, TK * SK * SK, DIM_OUT))

    xpad = np.zeros((B, DIM, T + TK - 1, HP, WP), np.float32)
    xpad[:, :, TK - 1:, 1:1 + H, 1:1 + W] = fmap

    in_maps = []
    for c in range(N_CORES):
        b, th = divmod(c, 2)
        t0 = th * TH
        in_maps.append({
            "xp": np.ascontiguousarray(xpad[b, :, t0:t0 + PF]),
            "wm": wmod[b],
        })
    return in_maps


def assemble_output(results):
    """Gather per-core [O, TH, H, W] results into [B, O, T, H, W]."""
    out = np.empty((B, DIM_OUT, T, H, W), np.float32)
    for c, r in enumerate(results):
        b, th = divmod(c, 2)
        out[b, :, th * TH:(th + 1) * TH] = r["y"]
    return out


_NC_CACHE = []


def kernel(**inputs):
    fmap = inputs["fmap"]
    cond = inputs["cond"]
    weights = inputs["weights"]
    in_maps = prepare_inputs(fmap, cond, weights)
    if not _NC_CACHE:
        _NC_CACHE.append(build_nc())
    nc = _NC_CACHE[0]
    res = run_bass_kernel_spmd(nc, in_maps, core_ids=list(range(N_CORES)))
    return assemble_output(res.results)


# revision 21
# speedup vs baseline: 228.6320x; 1.0190x over previous
"""Trainium2 Bass kernel for per-sample modulated causal Conv3D.

Reference semantics (see problem):
  w[b,o,i,kt,kh,kw] = W[o,i,kt,kh,kw] * (cond[b,i] + 1)
  w /= sqrt(max(sum_{i,kt,kh,kw} w^2, eps))        (per b,o)
  y[b] = conv3d(pad(fmap[b], t:(2,0), h:(1,1), w:(1,1)), w[b])  VALID

Shapes: fmap [4,128,16,64,64] f32, cond [4,128], W [128,128,3,3,3].

Strategy: weight modulation/demodulation is a tiny O(1.7MB) elementwise
computation done on host in float64; the 232 GFLOP conv runs on 8
NeuronCores. Sharding: 8 shards = (batch b in 0..3) x (time half in 0..1),
each core computes its sample's full 128 output channels for 8 output
frames. The conv is expressed as 27 accumulated matmuls per PSUM bank:
stationary operand = per-tap modulated weights [i=128, o=128], moving
operand = spatially-padded input frame window [i=128, 8 rows, 64 cols]
(N=512 = one PSUM bank).

Matmul inputs use dtype float32r (full PE rate at N>=512 vs 1/4 rate for
plain float32).
"""

import numpy as np
import orjson

import concourse.bass as bass
import concourse.mybir as mybir
import concourse.tile as tile
from concourse.bass_utils import run_bass_kernel_spmd
from concourse.vector_clock import ScopedClock, VectorClock

# Problem constants (hardcoded per harness contract).
B = 4
DIM = 128
DIM_OUT = 128
T = 16
H = 64
W = 64
TK = 3
SK = 3
EPS = 1e-8
N_CORES = 8
TH = T // 2  # frames per core (time half)
PF = TH + TK - 1  # padded frames per core
HP = H + 2  # spatially padded rows
WP = W + 2  # spatially padded cols (live data)
WPA = WP  # allocated row pitch; 68 would make rows 16B-aligned in SBUF
ROWS_PER_BANK = 8  # 8 rows x 64 cols = 512 = one fp32 PSUM bank
N_BANKS = H // ROWS_PER_BANK
BANK_GROUP = 4  # banks accumulated concurrently per group

DT_MM = mybir.dt.float32r  # matmul input dtype (4-byte storage, fp32 data)


class _TileContextChunkedDrain(tile.TileContext):
    """TileContext whose tail drain splits its sem waits across a chain of
    sync NOPs (1 wait each): the walrus build in this container rejects CTRL
    instructions carrying more than one sync-wait command."""

    def _drain_and_barrier(self, tick_clock, wait_clock):
        vec = list(tick_clock.global_clock)
        for i, t in enumerate(vec):
            if t <= 0:
                continue
            v = [0] * len(vec)
            v[i] = t
            nop = self.nc.sync.nop()
            wait_clock.add_sem_waits(nop.ins, ScopedClock({None: VectorClock(v)}))
        self.nc.sync.drain()
        self.nc.all_engine_barrier()
        assert self.sems is not None
        popped = self.nc._tile_sem_poison_stack.pop()
        assert popped is self._sem_poison
        self.nc.clear_and_free_semaphores(list(self.sems.allocated().values()))
        self.nc.all_engine_barrier()


def _split_multi_waits(bir: bytes) -> bytes:
    """The walrus build here rejects instructions carrying more than one
    sync-wait command. Un-fuse: move extra waits onto NoOps inserted just
    before the instruction on the same engine queue (strictly equivalent —
    the engine queue stalls on the NoOp's wait first)."""
    m = orjson.loads(bir)
    ctr = 0
    for f in m["functions"]:
        for bb in f["blocks"]:
            out = []
            for inst in bb["instructions"]:
                si = inst.get("sync_info")
                waits = (si or {}).get("on_wait") or []
                if len(waits) > 1:
                    for w in waits[:-1]:
                        ctr += 1
                        out.append({
                            "debug": inst.get("debug", 0),
                            "engine": inst["engine"],
                            "ins": [],
                            "outs": [],
                            "name": f"I-wsplit{ctr}",
                            "opcode": "NoOp",
                            "sync_info": {"on_update": [], "on_wait": [w]},
                        })
                    si["on_wait"] = [waits[-1]]
                out.append(inst)
            bb["instructions"] = out
    return orjson.dumps(m)


def _patch_to_json_bytes(nc):
    """Route every serialization of this module through _split_multi_waits."""
    orig = nc.to_json_bytes

    def to_json_bytes():
        return _split_multi_waits(orig())

    nc.to_json_bytes = to_json_bytes
    return nc


def build_nc(repeats=1, bank_group=BANK_GROUP, warmup_mms=24, xbufs=4,
             wpa=WPA):
    """Build the per-core SPMD Bass program (identical on all cores).

    repeats>1 re-runs the whole body (loads + conv + stores) that many
    times — used only for slope-based HW timing (axon hides NTFF profiling,
    so per-NEFF wall-clock can't resolve sub-ms kernel times)."""
    nc = bass.Bass("TRN2", target_bir_lowering=False, debug=False,
                   num_devices=N_CORES)
    xp = nc.dram_tensor("xp", [DIM, PF, HP, wpa], DT_MM, kind="ExternalInput")
    wm = nc.dram_tensor("wm", [DIM, TK * SK * SK, DIM_OUT], DT_MM,
                        kind="ExternalInput")
    y = nc.dram_tensor("y", [DIM_OUT, TH, H, W], mybir.dt.float32,
                       kind="ExternalOutput")

    with _TileContextChunkedDrain(nc) as tc:
        psum_bufs = min(8, 2 * bank_group)
        with (
            tc.tile_pool(name="wpool", bufs=1) as wpool,
            tc.tile_pool(name="xpool", bufs=xbufs) as xpool,
            tc.tile_pool(name="ypool", bufs=2) as ypool,
            tc.tile_pool(name="ppool", bufs=psum_bufs, space="PSUM") as ppool,
        ):
            if warmup_mms:
                # Keep the PE busy during the initial DMA window so the HAM
                # clock gate reaches 8/8 before the real matmuls start.
                wu_w = wpool.tile([DIM, DIM_OUT], mybir.dt.float32, name="wu_w")
                wu_x = wpool.tile([DIM, ROWS_PER_BANK * W], mybir.dt.float32,
                                  name="wu_x")
                nc.gpsimd.memset(wu_w[:], 0.0)
                nc.gpsimd.memset(wu_x[:], 0.0)
                wu_ps = ppool.tile([DIM_OUT, ROWS_PER_BANK * W],
                                   mybir.dt.float32, name="wu_ps", tag="psum")
                for i in range(warmup_mms):
                    nc.tensor.matmul(wu_ps[:], wu_w[:].bitcast(DT_MM),
                                     wu_x[:].bitcast(DT_MM),
                                     start=True, stop=True)

            wt = wpool.tile([DIM, TK * SK * SK, DIM_OUT], DT_MM)
            frames = {}

            def load_frame(fi, rep=0, chunks=1):
                xt = xpool.tile([DIM, HP, wpa], DT_MM, name=f"x_r{rep}f{fi}",
                                tag="xframe")
                if chunks == 1:
                    nc.sync.dma_start(xt[:], xp.ap()[:, fi])
                else:
                    # startup-critical loads: row-chunks across parallel
                    # DMA queues to cut first-matmul latency
                    step = (HP + chunks - 1) // chunks
                    for r0 in range(0, HP, step):
                        r1 = min(r0 + step, HP)
                        nc.sync.dma_start(xt[:, r0:r1], xp.ap()[:, fi, r0:r1])
                frames[fi] = xt

            # Startup order matters: the first matmuls gate on wt[kt=0] and
            # frame 0, so interleave their chunks across the first DMA queue
            # slots; everything else queues behind them.
            xt0 = xpool.tile([DIM, HP, wpa], DT_MM, name="x_r0f0", tag="xframe")
            for c in range(SK):
                ws = slice(c * SK, (c + 1) * SK)
                nc.sync.dma_start(wt[:, ws], wm.ap()[:, ws])
                r0, r1 = c * 22, (c + 1) * 22
                nc.sync.dma_start(xt0[:, r0:r1], xp.ap()[:, 0, r0:r1])
            frames[0] = xt0
            for kt in range(1, TK):
                s = slice(kt * SK * SK, (kt + 1) * SK * SK)
                nc.sync.dma_start(wt[:, s], wm.ap()[:, s])

            for rep in range(repeats):
              for fi in range(TK - 1):
                if (rep, fi) == (0, 0):
                    continue  # issued above
                load_frame(fi, rep, chunks=4 if rep == 0 else 1)

              for f in range(TH):
                load_frame(f + TK - 1, rep,
                           chunks=4 if (rep == 0 and f == 0) else 1)
                yt = ypool.tile([DIM_OUT, H, W], mybir.dt.float32, tag="yframe")
                for g in range(N_BANKS // bank_group):
                    pts = [
                        ppool.tile([DIM_OUT, ROWS_PER_BANK, W], mybir.dt.float32,
                                   name=f"ps_r{rep}f{f}g{g}b{jj}", tag="psum")
                        for jj in range(bank_group)
                    ]
                    for kt in range(TK):
                        xt = frames[f + kt]
                        for kh in range(SK):
                            for kw in range(SK):
                                tap = kt * SK * SK + kh * SK + kw
                                for jj in range(bank_group):
                                    j = g * bank_group + jj
                                    r0 = ROWS_PER_BANK * j + kh
                                    nc.tensor.matmul(
                                        pts[jj][:],
                                        wt[:, tap],
                                        xt[:, r0:r0 + ROWS_PER_BANK, kw:kw + W],
                                        start=(tap == 0),
                                        stop=(tap == TK * SK * SK - 1),
                                    )
                    for jj in range(bank_group):
                        j = g * bank_group + jj
                        nc.vector.tensor_copy(
                            yt[:, ROWS_PER_BANK * j:ROWS_PER_BANK * (j + 1), :],
                            pts[jj][:],
                        )
                    # store per bank-group so the kernel tail only waits on
                    # the last group's rows, not a whole frame; the very last
                    # group stores per-bank to shrink the tail further
                    last = (rep == repeats - 1 and f == TH - 1
                            and g == N_BANKS // bank_group - 1)
                    if last:
                        for jj in range(bank_group):
                            j = g * bank_group + jj
                            br = slice(ROWS_PER_BANK * j, ROWS_PER_BANK * (j + 1))
                            nc.sync.dma_start(y.ap()[:, f, br], yt[:, br])
                    else:
                        gr = slice(ROWS_PER_BANK * g * bank_group,
                                   ROWS_PER_BANK * (g + 1) * bank_group)
                        nc.sync.dma_start(y.ap()[:, f, gr], yt[:, gr])
                del frames[f]
    return _patch_to_json_bytes(nc)


def modulate_weights(weights, cond):
    """Host-side weight modulation + demodulation (float64, cast to f32)."""
    w = weights.astype(np.float64)[None] * (cond.astype(np.float64)[:, None, :,
                                                                    None, None,
                                                                    None] + 1.0)
    ssq = np.sum(w * w, axis=(2, 3, 4, 5), keepdims=True)
    w = w / np.sqrt(np.maximum(ssq, EPS))
    return w.astype(np.float32)  # [B, O, I, TK, SK, SK]


def prepare_inputs(fmap, cond, weights, wpa=WPA):
    """Shard full inputs into per-core input maps."""
    fmap = np.asarray(fmap, dtype=np.float32)
    cond = np.asarray(cond, dtype=np.float32)
    weights = np.asarray(weights, dtype=np.float32)

    wmod = modulate_weights(weights, cond)
    # [B, O, I, kt, kh, kw] -> [B, I, (kt kh kw), O] so lhsT[i, tap, o].
    wmod = np.ascontiguousarray(wmod.transpose(0, 2, 3, 4, 5, 1).reshape(
        B, DIM, TK * SK * SK, DIM_OUT))

    xpad = np.zeros((B, DIM, T + TK - 1, HP, wpa), np.float32)
    xpad[:, :, TK - 1:, 1:1 + H, 1:1 + W] = fmap

    in_maps = []
    for c in range(N_CORES):
        b, th = divmod(c, 2)
        t0 = th * TH
        in_maps.append({
            "xp": np.ascontiguousarray(xpad[b, :, t0:t0 + PF]),
            "wm": wmod[b],
        })
    return in_maps


def assemble_output(results):
    """Gather per-core [O, TH, H, W] results into [B, O, T, H, W]."""
    out = np.empty((B, DIM_OUT, T, H, W), np.float32)
    for c, r in enumerate(results):
        b, th = divmod(c, 2)
        out[b, :, th * TH:(th + 1) * TH] = r["y"]
    return out


_NC_CACHE = []


def kernel(**inputs):
    fmap = inputs["fmap"]
    cond = inputs["cond"]
    weights = inputs["weights"]
    in_maps = prepare_inputs(fmap, cond, weights)
    if not _NC_CACHE:
        _NC_CACHE.append(build_nc())
    nc = _NC_CACHE[0]
    res = run_bass_kernel_spmd(nc, in_maps, core_ids=list(range(N_CORES)))
    return assemble_output(res.results)
